# revision 3
# baseline (speedup 1.0000x reference)
"""Trainium2 Bass kernel for nn_AttentionLayer — v2.

Changes vs baseline (423us):
  - AV matmuls col-packed: head A -> psum partitions 0-63 (col grps 0,1),
    head B -> 64-127 (grps 2,3), concurrent; denominators via 4 concurrent
    M=1 ones-matmuls (col grps 0..3) every 2nd s-tile.  AV PE 110->83us.
  - softmax denominators: [2,512] -> DMA-transpose -> [128,8] -> one 190ns
    DVE reciprocal (was 32x 3.3us single-lane reciprocals).
  - softmax scale folded into exp's free affine; q/k PSUM->SBUF bias
    copies moved from ACT (the floor engine, ~285us of exp) to DVE.
  - prologue: k/q weights+inputs DMA'd first (per-l-chunk tiles), v
    projected from s-chunked DMAs dripped inside pair-0's attention loop.
  - fewer, larger DMAs (combined weight / per-lc input tiles).
"""

from collections import deque

import numpy as np

import concourse.bass as bass
import concourse.mybir as mybir
import concourse.tile as tile
from concourse import bacc
from concourse.bass_utils import run_bass_kernel_spmd

F32 = mybir.dt.float32
EXP = mybir.ActivationFunctionType.Exp
ADD = mybir.AluOpType.add
MULT = mybir.AluOpType.mult

E = 64          # head dim
J = 16          # mix factor: total heads in the reference model
JE = J * E      # 1024 rows of Wo

MM_DTS = {
    "f32r": mybir.dt.float32r,
    "f32": mybir.dt.float32,
    "bf16": mybir.dt.bfloat16,
}


def build_core_kernel(L=2048, D=1024, NH=8, OUT_D=1024, mm_dt="bf16"):
    """Builds the per-core Bacc graph (SPMD: all 8 cores run this)."""
    HE = NH * E               # projected width per core
    HEC = HE // 128           # qT/kT e-chunks (head pairs)
    NHP = NH // 2             # head pairs
    KC = D // 128             # contraction tiles for projections
    R = L // J                # output rows per head
    LCH = min(512, L)         # l-chunk
    NLC = L // LCH
    NST = L // 128            # s-tiles
    DCH = min(512, OUT_D)     # out-proj n-chunk
    NDC = OUT_D // DCH
    WOC = JE // 128           # Wo contraction chunks
    SCALE = 1.0 / np.sqrt(E)
    assert L % J == 0 and R <= 128 and HE % 128 == 0 and NST % 2 == 0

    MDT = MM_DTS[mm_dt]

    nc = bacc.Bacc("TRN2", target_bir_lowering=False, debug=False,
                   enable_asserts=False)

    qT_ext = nc.declare_dram_parameter("qT", [D, L], MDT, isOutput=False)
    kT_ext = nc.declare_dram_parameter("kT", [D, L], MDT, isOutput=False)
    vT_ext = nc.declare_dram_parameter("vT", [D, L], MDT, isOutput=False)
    wq_ext = nc.declare_dram_parameter("wq", [D, HE], MDT, isOutput=False)
    wk_ext = nc.declare_dram_parameter("wk", [D, HE], MDT, isOutput=False)
    wv_ext = nc.declare_dram_parameter("wv", [D, HE], MDT, isOutput=False)
    bq_ext = nc.declare_dram_parameter("bq", [HE], F32, isOutput=False)
    bk_ext = nc.declare_dram_parameter("bk", [HE], F32, isOutput=False)
    bv_ext = nc.declare_dram_parameter("bv", [HE], F32, isOutput=False)
    wo_ext = nc.declare_dram_parameter("wo", [JE, OUT_D], MDT, isOutput=False)
    bo_ext = nc.declare_dram_parameter("bo", [OUT_D], F32, isOutput=False)
    out_ext = nc.declare_dram_parameter("out", [NH * R, OUT_D], F32,
                                        isOutput=True)

    with tile.TileContext(nc) as tc:
        with (
            tc.tile_pool(name="const", bufs=1) as const,
            tc.tile_pool(name="kqin", bufs=1) as kqin,
            tc.tile_pool(name="vtp", bufs=3) as vtp,
            tc.tile_pool(name="acts", bufs=1) as acts,
            tc.tile_pool(name="expp", bufs=4) as expp,
            tc.tile_pool(name="attnd", bufs=3) as attnd,
            tc.tile_pool(name="pcpp", bufs=2) as pcpp,
            tc.tile_pool(name="epi", bufs=1) as epi,
            tc.tile_pool(name="outp", bufs=2) as outp,
        ):
            # ---- tiny constants first (cheap DMAs) ----
            bqt = const.tile([128, HEC], F32, tag="bqt")
            nc.sync.dma_start(bqt[:], bq_ext.rearrange("(c p) -> p c", p=128))
            bkt = const.tile([128, HEC], F32, tag="bkt")
            nc.sync.dma_start(bkt[:], bk_ext.rearrange("(c p) -> p c", p=128))

            bv_row = const.tile([1, HE], F32, tag="bv_row")
            nc.sync.dma_start(bv_row[:],
                              bv_ext.rearrange("(o he) -> o he", o=1))
            bv_bc = const.tile([128, HE], F32, tag="bv_bc")
            nc.gpsimd.partition_broadcast(bv_bc[:], bv_row[:], channels=128)

            bo_row = const.tile([1, OUT_D], F32, tag="bo_row")
            nc.sync.dma_start(bo_row[:],
                              bo_ext.rearrange("(o d) -> o d", o=1))
            bo_bc = const.tile([128, OUT_D], F32, tag="bo_bc")
            nc.gpsimd.partition_broadcast(bo_bc[:], bo_row[:], channels=128)

            ones1 = const.tile([128, 1], MDT, tag="ones1")
            nc.vector.memset(ones1[:], 1.0)

            # ---- combined weight tiles: w[:, dt*HE + he] = w_ext[dt*128+p, he]
            def load_w_combined(w_ext, tag):
                w = const.tile([128, KC * HE], MDT, tag=tag)
                nc.sync.dma_start(
                    w.rearrange("p (dt he) -> p dt he", dt=KC),
                    w_ext.rearrange("(dt p) he -> p dt he", p=128))
                return w

            # per-lc input tiles: x[p, dt*LCH + l] = xT_ext[dt*128+p, lc*LCH+l]
            def load_x_lc(in_ext, pfx, lc):
                x = kqin.tile([128, KC * LCH], MDT, tag=f"{pfx}{lc}")
                nc.sync.dma_start(
                    x.rearrange("p (dt l) -> p dt l", dt=KC),
                    in_ext.rearrange("(dt p) l -> p dt l",
                                     p=128)[:, :, lc * LCH:(lc + 1) * LCH])
                return x

            # DMA issue order = priority order.  The k side needs ALL
            # l-chunks before the first st-sweep of scores (scores(lc, st)
            # reads kT_sb[:, st*128...] across the whole of L); the q side
            # only needs lc0/lc1 early (scores(lc) reads qT_sb's lc chunk).
            wkt = load_w_combined(wk_ext, "wk")
            kin = [load_x_lc(kT_ext, "xk", lc) for lc in range(NLC)]
            wqt = load_w_combined(wq_ext, "wq")
            qin = [load_x_lc(qT_ext, "xq", lc)
                   for lc in range(min(2, NLC))]
            wvt = load_w_combined(wv_ext, "wv")
            # vt chunk st: vt[p, dt*128 + s] = vT_ext[dt*128+p, st*128+s]
            vt_view = vT_ext.rearrange("(dt p) l -> p dt l", p=128)

            def load_vt(st):
                v = vtp.tile([128, KC * 128], MDT, tag="vt")
                nc.sync.dma_start(
                    v.rearrange("p (dt s) -> p dt s", dt=KC),
                    vt_view[:, :, st * 128:(st + 1) * 128])
                return v

            vt_tiles = [load_vt(st) for st in range(NST)]
            for lcx in range(min(2, NLC), NLC):
                qin.append(load_x_lc(qT_ext, "xq", lcx))

            wo_sb = const.tile([128, WOC * OUT_D], MDT, tag="wo")
            nc.sync.dma_start(
                wo_sb.rearrange("p (t od) -> p t od", t=WOC),
                wo_ext.rearrange("(t p) od -> p t od", p=128))

            # projected q/k: e-chunk hp lives in slot hp % 2
            qT_sb = [acts.tile([128, L], MDT, tag=f"qTs{i % 2}",
                               name=f"qT_sb{i}") for i in range(HEC)]
            kT_sb = [acts.tile([128, L], MDT, tag=f"kTs{i % 2}",
                               name=f"kT_sb{i}") for i in range(HEC)]
            # projected v (+bias): v_sb[st] is [128 s, HE]
            v_sb = [acts.tile([128, HE], MDT, tag=f"vsb{st}",
                              name=f"v_sb{st}") for st in range(NST)]

            with (
                tc.tile_pool(name="psc", bufs=2, space="PSUM") as psc,
                tc.tile_pool(name="pacc", bufs=4, space="PSUM") as pacc,
            ):
                def proj_gen(wt, xin, dest, bcol, hp, lcs):
                    """k/q projection of e-chunk hp for the given lcs."""
                    for lc in lcs:
                        psq = pacc.tile([128, LCH], F32, tag="acc",
                                        name="psq")
                        for dt in range(KC):
                            nc.tensor.matmul(
                                psq[:],
                                wt[:, dt * HE + hp * 128:
                                   dt * HE + (hp + 1) * 128],
                                xin[lc][:, dt * LCH:(dt + 1) * LCH],
                                start=(dt == 0), stop=(dt == KC - 1))
                            yield
                        # PSUM->SBUF copy + per-partition bias on DVE
                        nc.vector.tensor_scalar(
                            dest[hp][:, lc * LCH:(lc + 1) * LCH],
                            psq[:], bcol[:, hp:hp + 1], None, ADD)
                        yield

                def vproj_st(st):
                    """project v for s-tile st: 8 MMs + bias add, inline."""
                    vps = pacc.tile([128, HE], F32, tag="acc", name="vps")
                    for dt in range(KC):
                        nc.tensor.matmul(
                            vps[:],
                            vt_tiles[st][:, dt * 128:(dt + 1) * 128],
                            wvt[:, dt * HE:(dt + 1) * HE],
                            start=(dt == 0), stop=(dt == KC - 1))
                    nc.vector.tensor_add(v_sb[st][:], vps[:], bv_bc[:])

                def outproj_gen(dups, hp):
                    for loc in range(2):
                        yield from outproj_one(dups[loc], 2 * hp + loc)

                def outproj_one(dup, h):
                    lhs = dup.rearrange("p (r j) -> p j r", j=J)
                    for dc in range(NDC):
                        po = pacc.tile([R, DCH], F32, tag="acc", name="po")
                        for t in range(WOC):
                            nc.tensor.matmul(
                                po[:],
                                lhs[:, 2 * t, :],
                                wo_sb[:, t * OUT_D + dc * DCH:
                                      t * OUT_D + (dc + 1) * DCH],
                                start=(t == 0), stop=(t == WOC - 1))
                            yield
                        ob = outp.tile([R, DCH], F32, tag="outp", name="ob")
                        nc.vector.tensor_add(
                            ob[:], po[:],
                            bo_bc[0:R, dc * DCH:(dc + 1) * DCH])
                        nc.sync.dma_start(
                            out_ext[h * R:(h + 1) * R,
                                    dc * DCH:(dc + 1) * DCH],
                            ob[:])
                        yield

                def run_gen(gen):
                    if gen is not None:
                        for _ in gen:
                            pass

                def epilogue(pav, dps, dups, lc, hp, tail=False):
                    """normalize lc's AV block; emitted at lc end,
                    self-paced via deps (recip is batched + cheap)."""
                    # quick-release copies (free the PSUM banks)
                    pcp = [pcpp.tile([64, LCH], F32, tag=f"pcp{i}",
                                     name="pcp") for i in range(2)]
                    nc.vector.tensor_copy(pcp[0][:], pav[0:64, :])
                    nc.vector.tensor_copy(pcp[1][:], pav[64:128, :])
                    dadd = [epi.tile([1, LCH], F32, tag=f"dadd{i}",
                                     name="dadd") for i in range(2)]
                    dtmp = [epi.tile([1, LCH], F32, tag=f"dtmp{i}",
                                     name="dtmp") for i in range(2)]
                    nc.vector.tensor_copy(dtmp[0][:], dps[0:1, :])
                    nc.vector.tensor_add(dadd[0][:], dtmp[0][:],
                                         dps[64:65, :])
                    nc.vector.tensor_copy(dtmp[1][:], dps[32:33, :])
                    nc.vector.tensor_add(dadd[1][:], dtmp[1][:],
                                         dps[96:97, :])
                    # [1,LCH] -> [64, LCH/64] so reciprocal runs 128 lanes
                    dT = epi.tile([128, LCH // 64], F32, tag="dT", name="dT")
                    nc.sync.dma_start(dT[0:64, :], dadd[0][:])
                    nc.sync.dma_start(dT[64:128, :], dadd[1][:])
                    rT = epi.tile([128, LCH // 64], F32, tag="rT", name="rT")
                    nc.vector.reciprocal(rT[:], dT[:])
                    for loc in range(2):
                        rrow = epi.tile([1, LCH], F32, tag=f"rrow{loc}",
                                        name="rrow")
                        nc.sync.dma_start(rrow[:],
                                          rT[loc * 64:(loc + 1) * 64, :])
                        bc = epi.tile([64, LCH], F32, tag=f"bc{loc}",
                                      name="bc")
                        nc.gpsimd.partition_broadcast(bc[:], rrow[:],
                                                      channels=64)
                        nc.vector.tensor_mul(
                            dups[loc][0:64, lc * LCH:(lc + 1) * LCH],
                            pcp[loc][:], bc[:])
                        if tail:
                            # last pair/lc: finish + project this head
                            # while the other head's chain still runs
                            finish_head(dups, loc, hp)
                            run_gen(outproj_one(dups[loc], 2 * hp + loc))

                def finish_head(dups, loc, hp):
                    nc.sync.dma_start(dups[loc][64:128, 0:L - 1],
                                      dups[loc][0:64, 1:L])

                # ---- prologue: project ALL of k (scores sweep the whole
                # of kT every lc), and q lc0/lc1, for pair 0
                gk = proj_gen(wkt, kin, kT_sb, bkt, 0, range(NLC))
                gq = proj_gen(wqt, qin, qT_sb, bqt, 0,
                              range(min(2, NLC)))
                alive = [gk, gq]
                while alive:
                    for g in list(alive):
                        try:
                            next(g)
                        except StopIteration:
                            alive.remove(g)

                pending = None
                for hp in range(NHP):
                    fill = deque()
                    if hp == 0 and NLC > 2:
                        fill.append(proj_gen(wqt, qin, qT_sb, bqt, 0,
                                             range(2, NLC)))
                    if hp + 1 < NHP:
                        fill.append(proj_gen(wkt, kin, kT_sb, bkt, hp + 1,
                                             range(NLC)))
                        fill.append(proj_gen(wqt, qin, qT_sb, bqt, hp + 1,
                                             range(NLC)))
                    if pending is not None:
                        fill.append(outproj_gen(*pending))
                        pending = None

                    def drain_fill(n, fill=fill):
                        while n > 0 and fill:
                            g = fill[0]
                            try:
                                next(g)
                                n -= 1
                            except StopIteration:
                                fill.popleft()

                    dups = [attnd.tile([128, L], MDT, tag="attnd",
                                       name="dup") for _ in range(2)]
                    W = 2 * LCH

                    def make_do_av(pav, dps, exq, lc):
                        def do_av(ex, st):
                            # col-packed AV: A -> partitions 0:64,
                            # B -> 64:128, concurrent
                            for loc in range(2):
                                h = 2 * hp + loc
                                nc.tensor.matmul(
                                    pav[loc * 64:(loc + 1) * 64, :],
                                    v_sb[st][:, h * E:(h + 1) * E],
                                    ex[:, loc * LCH:(loc + 1) * LCH],
                                    start=(st == 0), stop=(st == NST - 1),
                                    skip_group_check=True)
                            exq.append(ex)
                            # denominator quad: 4 concurrent M=1 matmuls
                            if st % 2 == 1:
                                q = st // 2
                                for i, (exi, half) in enumerate(
                                        ((exq[0], 0), (exq[0], 1),
                                         (exq[1], 0), (exq[1], 1))):
                                    nc.tensor.matmul(
                                        dps[32 * i:32 * i + 1, :],
                                        ones1[:],
                                        exi[:, half * LCH:
                                            (half + 1) * LCH],
                                        start=(q == 0),
                                        stop=(q == NST // 2 - 1),
                                        skip_group_check=True,
                                        tile_position=(0, 32 * i))
                                exq.clear()
                            if st == NST - 1:
                                epilogue(pav, dps, dups, lc, hp,
                                         tail=(hp + 1 == NHP
                                               and lc + 1 == NLC))
                        return do_av

                    # cross-lc software pipeline: the AV for st flushes
                    # during iteration st+2 (even across the lc boundary)
                    # so the score/exp stream is never gated by the
                    # exp-blocked AV/quad matmuls in the in-order PE queue
                    pend = deque()
                    for lc in range(NLC):
                        pav = pacc.tile([128, LCH], F32, tag="acc",
                                        name="pav")
                        dps = pacc.tile([128, LCH], F32, tag="acc",
                                        name="dps")
                        do_av = make_do_av(pav, dps, [], lc)
                        for st in range(NST):
                            sc = psc.tile([128, W], F32, tag="psc",
                                          name="sc")
                            for loc in range(2):
                                p0 = loc * 64
                                nc.tensor.matmul(
                                    sc[:, loc * LCH:(loc + 1) * LCH],
                                    kT_sb[hp][p0:p0 + 64,
                                              st * 128:(st + 1) * 128],
                                    qT_sb[hp][p0:p0 + 64,
                                              lc * LCH:(lc + 1) * LCH],
                                    start=True, stop=True)
                            ex = expp.tile([128, W], MDT, tag="exp",
                                           name="ex")
                            nc.scalar.activation(ex[:], sc[:], EXP,
                                                 scale=float(SCALE))
                            # v-projection dripped inside pair0/lc0
                            if hp == 0 and lc == 0:
                                vproj_st(st)
                            pend.append((do_av, ex, st))
                            while len(pend) > 2:
                                f, ex2, st2 = pend.popleft()
                                f(ex2, st2)
                            skip = (hp == 0 and lc == 0) or \
                                   (hp + 1 == NHP and lc == 0 and st < 8) \
                                   or st < 2 or st >= NST - 2
                            if not skip:
                                drain_fill(3)
                        # (pend intentionally carries into the next lc)
                    while pend:
                        f, ex2, st2 = pend.popleft()
                        f(ex2, st2)
                    # force-drain any leftover fill work (correctness:
                    # undrained generators = unemitted instructions)
                    while fill:
                        run_gen(fill.popleft())
                    if hp + 1 < NHP:
                        for loc in range(2):
                            finish_head(dups, loc, hp)
                        pending = (dups, hp)
                if pending is not None:
                    run_gen(outproj_gen(*pending))

    nc.compile()
    return nc


# ---------------------------------------------------------------------------
# host side
# ---------------------------------------------------------------------------

_NC_CACHE = {}

FULL_KEY = (2048, 1024, 8, 1024, "bf16")


def _get_nc(key=FULL_KEY):
    if key not in _NC_CACHE:
        _NC_CACHE[key] = build_core_kernel(*key)
    return _NC_CACHE[key]


def _np_mm_dtype(mm_dt):
    if mm_dt == "bf16":
        import ml_dtypes
        return ml_dtypes.bfloat16
    return np.float32


def make_in_maps(queries, keys, values, Wq, bq, Wk, bk, Wv, bv, Wo, bo,
                 mm_dt="bf16"):
    """Shard: core c handles batch c//2, heads NH*(c%2) .. NH*(c%2)+NH."""
    f = np.float32
    md = _np_mm_dtype(mm_dt)
    half_w = np.asarray(Wq).shape[1] // 2
    in_maps = []
    for c in range(8):
        b, half = c // 2, c % 2
        cs = slice(half * half_w, (half + 1) * half_w)
        in_maps.append({
            "qT": np.ascontiguousarray(np.asarray(queries[b], f).T.astype(md)),
            "kT": np.ascontiguousarray(np.asarray(keys[b], f).T.astype(md)),
            "vT": np.ascontiguousarray(np.asarray(values[b], f).T.astype(md)),
            "wq": np.ascontiguousarray(np.asarray(Wq, f)[:, cs].astype(md)),
            "wk": np.ascontiguousarray(np.asarray(Wk, f)[:, cs].astype(md)),
            "wv": np.ascontiguousarray(np.asarray(Wv, f)[:, cs].astype(md)),
            "bq": np.ascontiguousarray(np.asarray(bq, f)[cs]),
            "bk": np.ascontiguousarray(np.asarray(bk, f)[cs]),
            "bv": np.ascontiguousarray(np.asarray(bv, f)[cs]),
            "wo": np.ascontiguousarray(np.asarray(Wo, f).astype(md)),
            "bo": np.ascontiguousarray(np.asarray(bo, f)),
        })
    return in_maps


def assemble_output(results, B=4, L=2048, OUT_D=1024):
    out = np.empty((B, L, OUT_D), np.float32)
    half_rows = L // 2
    for c in range(8):
        b, half = c // 2, c % 2
        out[b, half * half_rows:(half + 1) * half_rows, :] = results[c]["out"]
    return out


def run_on_hw(inputs, trace=False, key=FULL_KEY, **kw):
    nc = _get_nc(key)
    in_maps = make_in_maps(**inputs, mm_dt=key[4])
    res = run_bass_kernel_spmd(nc, in_maps, core_ids=list(range(8)),
                               trace=trace, **kw)
    return assemble_output(res.results), res


def kernel(**inputs) -> np.ndarray:
    out, _ = run_on_hw(inputs, trace=False)
    return out


# revision 4
# speedup vs baseline: 1.0104x; 1.0104x over previous
"""Trainium2 Bass kernel for nn_AttentionLayer — v2.

Changes vs baseline (423us):
  - AV matmuls col-packed: head A -> psum partitions 0-63 (col grps 0,1),
    head B -> 64-127 (grps 2,3), concurrent; denominators via 4 concurrent
    M=1 ones-matmuls (col grps 0..3) every 2nd s-tile.  AV PE 110->83us.
  - softmax denominators: [2,512] -> DMA-transpose -> [128,8] -> one 190ns
    DVE reciprocal (was 32x 3.3us single-lane reciprocals).
  - softmax scale folded into exp's free affine; q/k PSUM->SBUF bias
    copies moved from ACT (the floor engine, ~285us of exp) to DVE.
  - prologue: k/q weights+inputs DMA'd first (per-l-chunk tiles), v
    projected from s-chunked DMAs dripped inside pair-0's attention loop.
  - fewer, larger DMAs (combined weight / per-lc input tiles).
"""

from collections import deque

import numpy as np

import concourse.bass as bass
import concourse.mybir as mybir
import concourse.tile as tile
from concourse import bacc
from concourse.bass_utils import run_bass_kernel_spmd

F32 = mybir.dt.float32
EXP = mybir.ActivationFunctionType.Exp
ADD = mybir.AluOpType.add
MULT = mybir.AluOpType.mult

E = 64          # head dim
J = 16          # mix factor: total heads in the reference model
JE = J * E      # 1024 rows of Wo

MM_DTS = {
    "f32r": mybir.dt.float32r,
    "f32": mybir.dt.float32,
    "bf16": mybir.dt.bfloat16,
}


def build_core_kernel(L=2048, D=1024, NH=8, OUT_D=1024, mm_dt="bf16"):
    """Builds the per-core Bacc graph (SPMD: all 8 cores run this)."""
    HE = NH * E               # projected width per core
    HEC = HE // 128           # qT/kT e-chunks (head pairs)
    NHP = NH // 2             # head pairs
    KC = D // 128             # contraction tiles for projections
    R = L // J                # output rows per head
    LCH = min(512, L)         # l-chunk
    NLC = L // LCH
    NST = L // 128            # s-tiles
    DCH = min(512, OUT_D)     # out-proj n-chunk
    NDC = OUT_D // DCH
    WOC = JE // 128           # Wo contraction chunks
    SCALE = 1.0 / np.sqrt(E)
    assert L % J == 0 and R <= 128 and HE % 128 == 0 and NST % 2 == 0

    MDT = MM_DTS[mm_dt]

    nc = bacc.Bacc("TRN2", target_bir_lowering=False, debug=False,
                   enable_asserts=False)

    qT_ext = nc.declare_dram_parameter("qT", [D, L], MDT, isOutput=False)
    kT_ext = nc.declare_dram_parameter("kT", [D, L], MDT, isOutput=False)
    vT_ext = nc.declare_dram_parameter("vT", [D, L], MDT, isOutput=False)
    wq_ext = nc.declare_dram_parameter("wq", [D, HE], MDT, isOutput=False)
    wk_ext = nc.declare_dram_parameter("wk", [D, HE], MDT, isOutput=False)
    wv_ext = nc.declare_dram_parameter("wv", [D, HE], MDT, isOutput=False)
    bq_ext = nc.declare_dram_parameter("bq", [HE], F32, isOutput=False)
    bk_ext = nc.declare_dram_parameter("bk", [HE], F32, isOutput=False)
    bv_ext = nc.declare_dram_parameter("bv", [HE], F32, isOutput=False)
    wo_ext = nc.declare_dram_parameter("wo", [JE, OUT_D], MDT, isOutput=False)
    bo_ext = nc.declare_dram_parameter("bo", [OUT_D], F32, isOutput=False)
    out_ext = nc.declare_dram_parameter("out", [NH * R, OUT_D], F32,
                                        isOutput=True)

    with tile.TileContext(nc) as tc:
        with (
            tc.tile_pool(name="const", bufs=1) as const,
            tc.tile_pool(name="kqin", bufs=1) as kqin,
            tc.tile_pool(name="vtp", bufs=3) as vtp,
            tc.tile_pool(name="acts", bufs=1) as acts,
            tc.tile_pool(name="expp", bufs=4) as expp,
            tc.tile_pool(name="attnd", bufs=3) as attnd,
            tc.tile_pool(name="pcpp", bufs=2) as pcpp,
            tc.tile_pool(name="epi", bufs=1) as epi,
            tc.tile_pool(name="outp", bufs=2) as outp,
        ):
            # ---- tiny constants first (cheap DMAs) ----
            bqt = const.tile([128, HEC], F32, tag="bqt")
            nc.sync.dma_start(bqt[:], bq_ext.rearrange("(c p) -> p c", p=128))
            bkt = const.tile([128, HEC], F32, tag="bkt")
            nc.sync.dma_start(bkt[:], bk_ext.rearrange("(c p) -> p c", p=128))

            bv_row = const.tile([1, HE], F32, tag="bv_row")
            nc.sync.dma_start(bv_row[:],
                              bv_ext.rearrange("(o he) -> o he", o=1))
            bv_bc = const.tile([128, HE], F32, tag="bv_bc")
            nc.gpsimd.partition_broadcast(bv_bc[:], bv_row[:], channels=128)

            bo_row = const.tile([1, OUT_D], F32, tag="bo_row")
            nc.sync.dma_start(bo_row[:],
                              bo_ext.rearrange("(o d) -> o d", o=1))
            bo_bc = const.tile([128, OUT_D], F32, tag="bo_bc")
            nc.gpsimd.partition_broadcast(bo_bc[:], bo_row[:], channels=128)

            ones1 = const.tile([128, 1], MDT, tag="ones1")
            nc.vector.memset(ones1[:], 1.0)

            # ---- combined weight tiles: w[:, dt*HE + he] = w_ext[dt*128+p, he]
            def load_w_combined(w_ext, tag):
                w = const.tile([128, KC * HE], MDT, tag=tag)
                nc.sync.dma_start(
                    w.rearrange("p (dt he) -> p dt he", dt=KC),
                    w_ext.rearrange("(dt p) he -> p dt he", p=128))
                return w

            # per-lc input tiles: x[p, dt*LCH + l] = xT_ext[dt*128+p, lc*LCH+l]
            def load_x_lc(in_ext, pfx, lc):
                x = kqin.tile([128, KC * LCH], MDT, tag=f"{pfx}{lc}")
                nc.sync.dma_start(
                    x.rearrange("p (dt l) -> p dt l", dt=KC),
                    in_ext.rearrange("(dt p) l -> p dt l",
                                     p=128)[:, :, lc * LCH:(lc + 1) * LCH])
                return x

            # DMA issue order = priority order.  The k side needs ALL
            # l-chunks before the first st-sweep of scores (scores(lc, st)
            # reads kT_sb[:, st*128...] across the whole of L); the q side
            # only needs lc0/lc1 early (scores(lc) reads qT_sb's lc chunk).
            wkt = load_w_combined(wk_ext, "wk")
            kin = [load_x_lc(kT_ext, "xk", 0)]
            wqt = load_w_combined(wq_ext, "wq")
            qin = [load_x_lc(qT_ext, "xq", 0)]
            for lcx in range(1, NLC):
                kin.append(load_x_lc(kT_ext, "xk", lcx))
            if NLC > 1:
                qin.append(load_x_lc(qT_ext, "xq", 1))
            wvt = load_w_combined(wv_ext, "wv")
            # vt chunk st: vt[p, dt*128 + s] = vT_ext[dt*128+p, st*128+s]
            vt_view = vT_ext.rearrange("(dt p) l -> p dt l", p=128)

            def load_vt(st):
                v = vtp.tile([128, KC * 128], MDT, tag="vt")
                nc.sync.dma_start(
                    v.rearrange("p (dt s) -> p dt s", dt=KC),
                    vt_view[:, :, st * 128:(st + 1) * 128])
                return v

            vt_tiles = [load_vt(st) for st in range(NST)]
            for lcx in range(min(2, NLC), NLC):
                qin.append(load_x_lc(qT_ext, "xq", lcx))

            wo_sb = const.tile([128, WOC * OUT_D], MDT, tag="wo")
            nc.sync.dma_start(
                wo_sb.rearrange("p (t od) -> p t od", t=WOC),
                wo_ext.rearrange("(t p) od -> p t od", p=128))

            # projected q/k: e-chunk hp lives in slot hp % 2
            qT_sb = [acts.tile([128, L], MDT, tag=f"qTs{i % 2}",
                               name=f"qT_sb{i}") for i in range(HEC)]
            kT_sb = [acts.tile([128, L], MDT, tag=f"kTs{i % 2}",
                               name=f"kT_sb{i}") for i in range(HEC)]
            # projected v (+bias): v_sb[st] is [128 s, HE]
            v_sb = [acts.tile([128, HE], MDT, tag=f"vsb{st}",
                              name=f"v_sb{st}") for st in range(NST)]

            with (
                tc.tile_pool(name="psc", bufs=2, space="PSUM") as psc,
                tc.tile_pool(name="pacc", bufs=4, space="PSUM") as pacc,
            ):
                def proj_gen(wt, xin, dest, bcol, hp, lcs):
                    """k/q projection of e-chunk hp for the given lcs."""
                    for lc in lcs:
                        psq = pacc.tile([128, LCH], F32, tag="acc",
                                        name="psq")
                        for dt in range(KC):
                            nc.tensor.matmul(
                                psq[:],
                                wt[:, dt * HE + hp * 128:
                                   dt * HE + (hp + 1) * 128],
                                xin[lc][:, dt * LCH:(dt + 1) * LCH],
                                start=(dt == 0), stop=(dt == KC - 1))
                            yield
                        # PSUM->SBUF copy + per-partition bias on DVE
                        nc.vector.tensor_scalar(
                            dest[hp][:, lc * LCH:(lc + 1) * LCH],
                            psq[:], bcol[:, hp:hp + 1], None, ADD)
                        yield

                def vproj_st(st):
                    """project v for s-tile st: 8 MMs + bias add, inline."""
                    vps = pacc.tile([128, HE], F32, tag="acc", name="vps")
                    for dt in range(KC):
                        nc.tensor.matmul(
                            vps[:],
                            vt_tiles[st][:, dt * 128:(dt + 1) * 128],
                            wvt[:, dt * HE:(dt + 1) * HE],
                            start=(dt == 0), stop=(dt == KC - 1))
                    nc.vector.tensor_add(v_sb[st][:], vps[:], bv_bc[:])

                def outproj_gen(dups, hp):
                    for loc in range(2):
                        yield from outproj_one(dups[loc], 2 * hp + loc)

                def outproj_one(dup, h):
                    lhs = dup.rearrange("p (r j) -> p j r", j=J)
                    for dc in range(NDC):
                        po = pacc.tile([R, DCH], F32, tag="acc", name="po")
                        for t in range(WOC):
                            nc.tensor.matmul(
                                po[:],
                                lhs[:, 2 * t, :],
                                wo_sb[:, t * OUT_D + dc * DCH:
                                      t * OUT_D + (dc + 1) * DCH],
                                start=(t == 0), stop=(t == WOC - 1))
                            yield
                        ob = outp.tile([R, DCH], F32, tag="outp", name="ob")
                        nc.vector.tensor_add(
                            ob[:], po[:],
                            bo_bc[0:R, dc * DCH:(dc + 1) * DCH])
                        nc.sync.dma_start(
                            out_ext[h * R:(h + 1) * R,
                                    dc * DCH:(dc + 1) * DCH],
                            ob[:])
                        yield

                def run_gen(gen):
                    if gen is not None:
                        for _ in gen:
                            pass

                def epilogue(pav, dps, dups, lc, hp, tail=False):
                    """normalize lc's AV block; emitted at lc end,
                    self-paced via deps (recip is batched + cheap)."""
                    # quick-release copies (free the PSUM banks)
                    pcp = [pcpp.tile([64, LCH], F32, tag=f"pcp{i}",
                                     name="pcp") for i in range(2)]
                    nc.vector.tensor_copy(pcp[0][:], pav[0:64, :])
                    nc.vector.tensor_copy(pcp[1][:], pav[64:128, :])
                    dadd = [epi.tile([1, LCH], F32, tag=f"dadd{i}",
                                     name="dadd") for i in range(2)]
                    dtmp = [epi.tile([1, LCH], F32, tag=f"dtmp{i}",
                                     name="dtmp") for i in range(2)]
                    nc.vector.tensor_copy(dtmp[0][:], dps[0:1, :])
                    nc.vector.tensor_add(dadd[0][:], dtmp[0][:],
                                         dps[64:65, :])
                    nc.vector.tensor_copy(dtmp[1][:], dps[32:33, :])
                    nc.vector.tensor_add(dadd[1][:], dtmp[1][:],
                                         dps[96:97, :])
                    # [1,LCH] -> [64, LCH/64] so reciprocal runs 128 lanes
                    dT = epi.tile([128, LCH // 64], F32, tag="dT", name="dT")
                    nc.sync.dma_start(dT[0:64, :], dadd[0][:])
                    nc.sync.dma_start(dT[64:128, :], dadd[1][:])
                    rT = epi.tile([128, LCH // 64], F32, tag="rT", name="rT")
                    nc.vector.reciprocal(rT[:], dT[:])
                    for loc in range(2):
                        rrow = epi.tile([1, LCH], F32, tag=f"rrow{loc}",
                                        name="rrow")
                        nc.sync.dma_start(rrow[:],
                                          rT[loc * 64:(loc + 1) * 64, :])
                        bc = epi.tile([64, LCH], F32, tag=f"bc{loc}",
                                      name="bc")
                        nc.gpsimd.partition_broadcast(bc[:], rrow[:],
                                                      channels=64)
                        nc.vector.tensor_mul(
                            dups[loc][0:64, lc * LCH:(lc + 1) * LCH],
                            pcp[loc][:], bc[:])
                        if tail:
                            # last pair/lc: finish + project this head
                            # while the other head's chain still runs
                            finish_head(dups, loc, hp)
                            run_gen(outproj_one(dups[loc], 2 * hp + loc))

                def finish_head(dups, loc, hp):
                    nc.sync.dma_start(dups[loc][64:128, 0:L - 1],
                                      dups[loc][0:64, 1:L])

                # ---- prologue: project ALL of k (scores sweep the whole
                # of kT every lc) and q lc0 for pair 0; q lc1 is emitted
                # inline late in lc0 (its first reader is lc1's scores)
                gk = proj_gen(wkt, kin, kT_sb, bkt, 0, range(NLC))
                gq = proj_gen(wqt, qin, qT_sb, bqt, 0, [0])
                alive = [gk, gq]
                while alive:
                    for g in list(alive):
                        try:
                            next(g)
                        except StopIteration:
                            alive.remove(g)

                pending = None
                for hp in range(NHP):
                    fill = deque()
                    if hp == 0 and NLC > 2:
                        fill.append(proj_gen(wqt, qin, qT_sb, bqt, 0,
                                             range(2, NLC)))
                    if hp + 1 < NHP:
                        fill.append(proj_gen(wkt, kin, kT_sb, bkt, hp + 1,
                                             range(NLC)))
                        fill.append(proj_gen(wqt, qin, qT_sb, bqt, hp + 1,
                                             range(NLC)))
                    if pending is not None:
                        fill.append(outproj_gen(*pending))
                        pending = None

                    def drain_fill(n, fill=fill):
                        while n > 0 and fill:
                            g = fill[0]
                            try:
                                next(g)
                                n -= 1
                            except StopIteration:
                                fill.popleft()

                    dups = [attnd.tile([128, L], MDT, tag="attnd",
                                       name="dup") for _ in range(2)]
                    W = 2 * LCH

                    def make_do_av(pav, dps, exq, lc):
                        def do_av(ex, st):
                            # col-packed AV: A -> partitions 0:64,
                            # B -> 64:128, concurrent
                            for loc in range(2):
                                h = 2 * hp + loc
                                nc.tensor.matmul(
                                    pav[loc * 64:(loc + 1) * 64, :],
                                    v_sb[st][:, h * E:(h + 1) * E],
                                    ex[:, loc * LCH:(loc + 1) * LCH],
                                    start=(st == 0), stop=(st == NST - 1),
                                    skip_group_check=True)
                            exq.append(ex)
                            # denominator quad: 4 concurrent M=1 matmuls
                            if st % 2 == 1:
                                q = st // 2
                                for i, (exi, half) in enumerate(
                                        ((exq[0], 0), (exq[0], 1),
                                         (exq[1], 0), (exq[1], 1))):
                                    nc.tensor.matmul(
                                        dps[32 * i:32 * i + 1, :],
                                        ones1[:],
                                        exi[:, half * LCH:
                                            (half + 1) * LCH],
                                        start=(q == 0),
                                        stop=(q == NST // 2 - 1),
                                        skip_group_check=True,
                                        tile_position=(0, 32 * i))
                                exq.clear()
                            if st == NST - 1:
                                epilogue(pav, dps, dups, lc, hp,
                                         tail=(hp + 1 == NHP
                                               and lc + 1 == NLC))
                        return do_av

                    # cross-lc software pipeline: the AV for st flushes
                    # during iteration st+2 (even across the lc boundary)
                    # so the score/exp stream is never gated by the
                    # exp-blocked AV/quad matmuls in the in-order PE queue
                    pend = deque()
                    for lc in range(NLC):
                        pav = pacc.tile([128, LCH], F32, tag="acc",
                                        name="pav")
                        dps = pacc.tile([128, LCH], F32, tag="acc",
                                        name="dps")
                        do_av = make_do_av(pav, dps, [], lc)
                        for st in range(NST):
                            sc = psc.tile([128, W], F32, tag="psc",
                                          name="sc")
                            for loc in range(2):
                                p0 = loc * 64
                                nc.tensor.matmul(
                                    sc[:, loc * LCH:(loc + 1) * LCH],
                                    kT_sb[hp][p0:p0 + 64,
                                              st * 128:(st + 1) * 128],
                                    qT_sb[hp][p0:p0 + 64,
                                              lc * LCH:(lc + 1) * LCH],
                                    start=True, stop=True)
                            ex = expp.tile([128, W], MDT, tag="exp",
                                           name="ex")
                            nc.scalar.activation(ex[:], sc[:], EXP,
                                                 scale=float(SCALE))
                            # v-projection dripped inside pair0/lc0
                            if hp == 0 and lc == 0:
                                vproj_st(st)
                                if NLC > 1 and st == max(0, NST - 3):
                                    run_gen(proj_gen(wqt, qin, qT_sb,
                                                     bqt, 0, [1]))
                            pend.append((do_av, ex, st))
                            while len(pend) > 2:
                                f, ex2, st2 = pend.popleft()
                                f(ex2, st2)
                            skip = (hp == 0 and lc == 0) or \
                                   (hp + 1 == NHP and lc == 0 and st < 8) \
                                   or st < 2 or st >= NST - 2
                            if not skip:
                                drain_fill(3)
                        # (pend intentionally carries into the next lc)
                    while pend:
                        f, ex2, st2 = pend.popleft()
                        f(ex2, st2)
                    # force-drain any leftover fill work (correctness:
                    # undrained generators = unemitted instructions)
                    while fill:
                        run_gen(fill.popleft())
                    if hp + 1 < NHP:
                        for loc in range(2):
                            finish_head(dups, loc, hp)
                        pending = (dups, hp)
                if pending is not None:
                    run_gen(outproj_gen(*pending))

    nc.compile()
    return nc


# ---------------------------------------------------------------------------
# host side
# ---------------------------------------------------------------------------

_NC_CACHE = {}

FULL_KEY = (2048, 1024, 8, 1024, "bf16")


def _get_nc(key=FULL_KEY):
    if key not in _NC_CACHE:
        _NC_CACHE[key] = build_core_kernel(*key)
    return _NC_CACHE[key]


def _np_mm_dtype(mm_dt):
    if mm_dt == "bf16":
        import ml_dtypes
        return ml_dtypes.bfloat16
    return np.float32


def make_in_maps(queries, keys, values, Wq, bq, Wk, bk, Wv, bv, Wo, bo,
                 mm_dt="bf16"):
    """Shard: core c handles batch c//2, heads NH*(c%2) .. NH*(c%2)+NH."""
    f = np.float32
    md = _np_mm_dtype(mm_dt)
    half_w = np.asarray(Wq).shape[1] // 2
    in_maps = []
    for c in range(8):
        b, half = c // 2, c % 2
        cs = slice(half * half_w, (half + 1) * half_w)
        in_maps.append({
            "qT": np.ascontiguousarray(np.asarray(queries[b], f).T.astype(md)),
            "kT": np.ascontiguousarray(np.asarray(keys[b], f).T.astype(md)),
            "vT": np.ascontiguousarray(np.asarray(values[b], f).T.astype(md)),
            "wq": np.ascontiguousarray(np.asarray(Wq, f)[:, cs].astype(md)),
            "wk": np.ascontiguousarray(np.asarray(Wk, f)[:, cs].astype(md)),
            "wv": np.ascontiguousarray(np.asarray(Wv, f)[:, cs].astype(md)),
            "bq": np.ascontiguousarray(np.asarray(bq, f)[cs]),
            "bk": np.ascontiguousarray(np.asarray(bk, f)[cs]),
            "bv": np.ascontiguousarray(np.asarray(bv, f)[cs]),
            "wo": np.ascontiguousarray(np.asarray(Wo, f).astype(md)),
            "bo": np.ascontiguousarray(np.asarray(bo, f)),
        })
    return in_maps


def assemble_output(results, B=4, L=2048, OUT_D=1024):
    out = np.empty((B, L, OUT_D), np.float32)
    half_rows = L // 2
    for c in range(8):
        b, half = c // 2, c % 2
        out[b, half * half_rows:(half + 1) * half_rows, :] = results[c]["out"]
    return out


def run_on_hw(inputs, trace=False, key=FULL_KEY, **kw):
    nc = _get_nc(key)
    in_maps = make_in_maps(**inputs, mm_dt=key[4])
    res = run_bass_kernel_spmd(nc, in_maps, core_ids=list(range(8)),
                               trace=trace, **kw)
    return assemble_output(res.results), res


def kernel(**inputs) -> np.ndarray:
    out, _ = run_on_hw(inputs, trace=False)
    return out
